# revision 16
# baseline (speedup 1.0000x reference)
"""Causal self-attention (B=1, T=4096, C=768, H=12) on 8 TRN2 NeuronCores.

Sharding: tensor-parallel over 4 head-groups (3 heads each) x 2 query-groups
(2048 queries each, causally balanced superblock assignment). Each core:
  - computes K^T/V^T for its 3 heads over the full sequence (x^T provided
    pre-transposed by the host),
  - computes scaled Q^T for its 2048 queries (host-gathered, rank-ordered),
  - runs causal flash attention in score-transposed (ST) layout: softmax
    denominators come free from a ones-column appended to V,
  - projects with its 192-row slice of w_proj, returning a partial y^T.
Host sums the 4 head-group partials per query-group, scatters the
superblocks back into sequence order and adds b_proj.
"""

import sys

sys.path.insert(0, "/opt/trn_rl_repo")

from contextlib import ExitStack

import numpy as np

import concourse.bass as bass
import concourse.tile as tile
from concourse import bacc, mybir
from concourse.bass_utils import run_bass_kernel_spmd

N_CORES = 8
T, C, H, HD = 4096, 768, 12, 64
HPC = 3              # heads per core (head-group size)
QSB = 256            # query superblock
NSB = T // QSB       # 16 global superblocks
R = 8                # ranks (superblocks per core)
KT = 128             # key tile
MASK_KT = 4          # last 4 key tiles of each rank carry the causal mask
NEG = -60.0          # additive mask value; exp(-60+8.5) ~ 4e-23

# Causally balanced superblock assignment per query-group, rank-sorted.
SB_QG = [
    [0, 2, 4, 6, 9, 11, 13, 15],
    [1, 3, 5, 7, 8, 10, 12, 14],
]
# Uniform per-rank key-tile bounds: max over query-groups of 2*(sb+1).
L_R = [max(2 * (SB_QG[0][r] + 1), 2 * (SB_QG[1][r] + 1)) for r in range(R)]

FP32 = mybir.dt.float32
FP32R = mybir.dt.float32r


def _build_program(debug_outputs=False):
    nc = bacc.Bacc("TRN2", target_bir_lowering=False, debug=False,
                   num_devices=N_CORES)

    xT = nc.dram_tensor("xT", [C, T], FP32, kind="ExternalInput").ap()
    xqT = nc.dram_tensor("xqT", [C, QSB * R], FP32, kind="ExternalInput").ap()
    wkv = nc.dram_tensor("wkv", [C, 2 * HPC * HD], FP32, kind="ExternalInput").ap()
    wq = nc.dram_tensor("wq", [C, HPC * HD], FP32, kind="ExternalInput").ap()
    wp = nc.dram_tensor("wp", [HPC * HD, C], FP32, kind="ExternalInput").ap()
    bkv = nc.dram_tensor("bkv", [3, 128, 1], FP32, kind="ExternalInput").ap()
    bq = nc.dram_tensor("bq", [2, 128, 1], FP32, kind="ExternalInput").ap()
    masks = nc.dram_tensor("masks", [R, 128, MASK_KT * QSB], FP32,
                           kind="ExternalInput").ap()
    ident = nc.dram_tensor("ident", [2 * HD, HD], FP32, kind="ExternalInput").ap()
    vones = nc.dram_tensor("vones", [128, T // KT], FP32, kind="ExternalInput").ap()
    yT = nc.dram_tensor("yT", [C, QSB * R], FP32, kind="ExternalOutput").ap()
    dbg = {}
    if debug_outputs:
        for nm, shp in [("d_kvt0", [128, T]), ("d_kvt1", [128, T]),
                        ("d_kvt2", [128, T]), ("d_qt0", [128, QSB * R]),
                        ("d_qt1", [64, QSB * R]),
                        ("d_vaug0", [128, (T // KT) * (HD + 1)]),
                        ("d_vaug1", [128, (T // KT) * (HD + 1)]),
                        ("d_vaug2", [128, (T // KT) * (HD + 1)]),
                        ("d_ont0", [128, QSB * R]), ("d_ont1", [64, QSB * R]),
                        ("d_dsb", [HD + 1, HPC * R * QSB])]:
            dbg[nm] = nc.dram_tensor(nm, shp, FP32, kind="ExternalOutput").ap()

    CB = C // 128        # 6 contraction blocks
    TCH = 512            # gemm T-chunk
    NTCH = T // TCH      # 8
    NQCH = QSB * R // TCH  # 4

    with tile.TileContext(nc) as tc, ExitStack() as ctx:
        consts = ctx.enter_context(tc.tile_pool(name="consts", bufs=1))
        xpool = ctx.enter_context(tc.tile_pool(name="xpool", bufs=2))
        persist = ctx.enter_context(tc.tile_pool(name="persist", bufs=1))
        ptp = ctx.enter_context(tc.tile_pool(name="ptp", bufs=3))
        mpool = ctx.enter_context(tc.tile_pool(name="mpool", bufs=2))
        rbp = ctx.enter_context(tc.tile_pool(name="rbp", bufs=3))
        dram = ctx.enter_context(tc.tile_pool(name="dram", bufs=1, space="DRAM"))
        psum = ctx.enter_context(tc.tile_pool(name="psum", bufs=2, space="PSUM"))

        # ---- constants into SBUF ----
        wkv_sb = consts.tile([128, CB, 2 * HPC * HD], FP32R, tag="wkv")
        nc.sync.dma_start(
            wkv_sb[:], wkv.rearrange("(a p) n -> p a n", p=128).bitcast(FP32R))
        wq_sb = consts.tile([128, CB, HPC * HD], FP32R, tag="wq")
        nc.sync.dma_start(
            wq_sb[:], wq.rearrange("(a p) n -> p a n", p=128).bitcast(FP32R))
        wp0_sb = consts.tile([128, C], FP32R, tag="wp0")
        nc.sync.dma_start(wp0_sb[:], wp[0:128, :].bitcast(FP32R))
        wp1_sb = consts.tile([64, C], FP32R, tag="wp1")
        nc.sync.dma_start(wp1_sb[:], wp[128:192, :].bitcast(FP32R))
        bkv_sb = [consts.tile([128, 1], FP32, tag=f"bkv{m}", name=f"bkv_sb{m}") for m in range(3)]
        for m in range(3):
            nc.sync.dma_start(bkv_sb[m][:], bkv[m])
        bq_sb = [consts.tile([128, 1], FP32, tag=f"bq{m}", name=f"bq_sb{m}") for m in range(2)]
        for m in range(2):
            nc.sync.dma_start(bq_sb[m][:], bq[m])
        ident_sb = consts.tile([2 * HD, HD], FP32R, tag="ident")
        nc.sync.dma_start(ident_sb[:], ident.bitcast(FP32R))

        # ---- persistent activations ----
        # K^T/V^T rows stacked [384, T] in 3 blocks of 128 partitions.
        kvt = [persist.tile([128, T], FP32R, tag=f"kvt{m}", name=f"kvt{m}") for m in range(3)]
        qt = [persist.tile([128, QSB * R], FP32R, tag="qt0", name="qt0"),
              persist.tile([64, QSB * R], FP32R, tag="qt1", name="qt1")]
        vaug = [persist.tile([128, (T // KT) * (HD + 1)], FP32R, tag=f"vaug{h}", name=f"vaug{h}")
                for h in range(HPC)]
        raw = [persist.tile([128, QSB * R], FP32, tag="raw0", name="raw0"),
               persist.tile([64, QSB * R], FP32, tag="raw1", name="raw1")]
        ont = [persist.tile([128, QSB * R], FP32R, tag="ont0", name="ont0"),
               persist.tile([64, QSB * R], FP32R, tag="ont1", name="ont1")]
        dscr = dram.tile([R, HPC * QSB], FP32, tag="dscr", name="dscr")

        add, mult = mybir.AluOpType.add, mybir.AluOpType.mult

        # ---- phase A2 first: scaled Q^T gemm (queries) ----
        for t in range(NQCH):
            xt = xpool.tile([128, CB, TCH], FP32R, tag="xt")
            nc.sync.dma_start(
                xt[:],
                xqT[:, t * TCH:(t + 1) * TCH]
                .rearrange("(a p) n -> p a n", p=128).bitcast(FP32R))
            for m in range(2):
                rows = 128 if m == 0 else 64
                ps = psum.tile([128, TCH], FP32, tag="mm")
                for cb in range(CB):
                    nc.tensor.matmul(
                        ps[:rows], wq_sb[:, cb, m * 128:m * 128 + rows],
                        xt[:, cb, :], start=(cb == 0), stop=(cb == CB - 1))
                nc.scalar.activation(
                    out=qt[m][:rows, t * TCH:(t + 1) * TCH], in_=ps[:rows],
                    func=mybir.ActivationFunctionType.Identity,
                    bias=bq_sb[m][:rows], scale=1.0 / np.sqrt(HD))

        # ones columns of V_aug, written once up front
        for h in range(HPC):
            ones_cols = vaug[h][:].rearrange(
                "p (k e) -> p k e", e=HD + 1)[:, :, HD:HD + 1]
            nc.sync.dma_start(
                ones_cols,
                vones.rearrange("p (k e) -> p k e", e=1).bitcast(FP32R))

        def kt_slice(h, kt):
            row = h * HD
            blk, off = row // 128, row % 128
            return kvt[blk][off:off + HD, kt * KT:(kt + 1) * KT]

        def qt_slice(h, r):
            row = h * HD
            blk, off = row // 128, row % 128
            return qt[blk][off:off + HD, r * QSB:(r + 1) * QSB]

        # ---- ranks: interleave K/V gemm chunk r, V-transpose, attention,
        # normalize and projection.  A t-chunk covers exactly the 4 key
        # tiles rank r adds over rank r-1, so every rank's inputs are ready
        # one step ahead and all engines pipeline across phases.
        for r in range(R):
            # K^T/V^T gemm for t-chunk r (key tiles 4r..4r+3)
            t = r
            xt = xpool.tile([128, CB, TCH], FP32R, tag="xt")
            nc.sync.dma_start(
                xt[:],
                xT[:, t * TCH:(t + 1) * TCH]
                .rearrange("(a p) n -> p a n", p=128).bitcast(FP32R))
            for m in range(3):
                ps = psum.tile([128, TCH], FP32, tag="mm")
                for cb in range(CB):
                    nc.tensor.matmul(
                        ps[:], wkv_sb[:, cb, m * 128:(m + 1) * 128],
                        xt[:, cb, :], start=(cb == 0), stop=(cb == CB - 1))
                nc.scalar.activation(
                    out=kvt[m][:, t * TCH:(t + 1) * TCH], in_=ps[:],
                    func=mybir.ActivationFunctionType.Identity,
                    bias=bkv_sb[m][:], scale=1.0)

            # V^T -> V natural for key tiles 4r..4r+3 (ones cols via DMA)
            for h in range(HPC):
                vrow = 2 * HPC * HD // 2 + h * HD
                blk, off = vrow // 128, vrow % 128
                ps = psum.tile([128, 4 * HD], FP32R, tag="mm")
                for j in range(4):
                    kt = 4 * r + j
                    nc.tensor.transpose(
                        ps[:, j * HD:(j + 1) * HD],
                        kvt[blk][off:off + HD, kt * KT:(kt + 1) * KT],
                        ident_sb[off:off + HD, :])
                dst = vaug[h][:, 4 * r * (HD + 1):(4 * r + 4) * (HD + 1)]
                dst = dst.rearrange("p (k e) -> p k e", e=HD + 1)[:, :, 0:HD]
                nc.vector.tensor_copy(
                    out=dst, in_=ps[:].rearrange("p (k e) -> p k e", e=HD))

            # attention for rank r
            L = L_R[r]
            nb = L // MASK_KT
            mask_sb = mpool.tile([128, MASK_KT * QSB], FP32, tag="mask")
            nc.sync.dma_start(mask_sb[:], masks[r])
            # per-rank denominator stage on partition HD (=64): DVE cannot
            # cross partitions, so denoms stay on the partition the PV
            # matmul wrote them to until the DRAM round-trip broadcast.
            dsbr = rbp.tile([HD + 1, HPC * QSB], FP32, tag="dsb", bufs=2)
            for h in range(HPC):
                ops = psum.tile([HD + 1, QSB], FP32, tag="o")
                for b in range(nb):
                    st = psum.tile([128, MASK_KT * QSB], FP32, tag="st")
                    for j in range(MASK_KT):
                        kt = MASK_KT * b + j
                        nc.tensor.matmul(
                            st[:, j * QSB:(j + 1) * QSB], kt_slice(h, kt),
                            qt_slice(h, r), start=True, stop=True)
                    if b == nb - 1:
                        nc.vector.tensor_tensor(
                            out=st[:], in0=st[:], in1=mask_sb[:], op=add)
                    pt = ptp.tile([128, MASK_KT * QSB], FP32R, tag="pt")
                    nc.scalar.activation(
                        out=pt[:], in_=st[:],
                        func=mybir.ActivationFunctionType.Exp)
                    for j in range(MASK_KT):
                        kt = MASK_KT * b + j
                        nc.tensor.matmul(
                            ops[:], vaug[h][:, kt * (HD + 1):(kt + 1) * (HD + 1)],
                            pt[:, j * QSB:(j + 1) * QSB],
                            start=(kt == 0), stop=(kt == L - 1))
                nc.vector.tensor_copy(
                    out=dsbr[HD:HD + 1, h * QSB:(h + 1) * QSB],
                    in_=ops[HD:HD + 1, :])
                row = h * HD
                blk, off = row // 128, row % 128
                nc.vector.tensor_copy(
                    out=raw[blk][off:off + HD, r * QSB:(r + 1) * QSB],
                    in_=ops[0:HD, :])

            # rank-level normalize + projection, pipelined behind rank r+1
            nc.sync.dma_start(dscr[r:r + 1, :], dsbr[HD:HD + 1, :])
            rb = rbp.tile([128, HPC * QSB], FP32, tag="rb", bufs=2)
            nc.sync.dma_start(
                rb[:], dscr[r:r + 1, :].partition_broadcast(128))
            rbr = rbp.tile([128, HPC * QSB], FP32, tag="rbr", bufs=2)
            nc.vector.reciprocal(rbr[:], rb[:])
            for h in range(HPC):
                row = h * HD
                blk, off = row // 128, row % 128
                nc.vector.tensor_tensor(
                    out=ont[blk][off:off + HD, r * QSB:(r + 1) * QSB],
                    in0=raw[blk][off:off + HD, r * QSB:(r + 1) * QSB],
                    in1=rbr[off:off + HD, h * QSB:(h + 1) * QSB], op=mult)
            for cb in range(CB):
                ps = psum.tile([128, QSB], FP32, tag="mm")
                nc.tensor.matmul(
                    ps[:], wp0_sb[:, cb * 128:(cb + 1) * 128],
                    ont[0][:, r * QSB:(r + 1) * QSB], start=True, stop=False)
                nc.tensor.matmul(
                    ps[:], wp1_sb[:, cb * 128:(cb + 1) * 128],
                    ont[1][:, r * QSB:(r + 1) * QSB], start=False, stop=True)
                ysb = rbp.tile([128, QSB], FP32, tag="ysb")
                nc.vector.tensor_copy(out=ysb[:], in_=ps[:])
                nc.sync.dma_start(
                    yT[cb * 128:(cb + 1) * 128, r * QSB:(r + 1) * QSB],
                    ysb[:])

        if debug_outputs:
            for m in range(3):
                nc.sync.dma_start(dbg[f"d_kvt{m}"], kvt[m][:].bitcast(FP32))
            nc.sync.dma_start(dbg["d_qt0"], qt[0][:].bitcast(FP32))
            nc.sync.dma_start(dbg["d_qt1"], qt[1][:].bitcast(FP32))
            for h in range(HPC):
                nc.sync.dma_start(dbg[f"d_vaug{h}"], vaug[h][:].bitcast(FP32))
            nc.sync.dma_start(dbg["d_ont0"], ont[0][:].bitcast(FP32))
            nc.sync.dma_start(dbg["d_ont1"], ont[1][:].bitcast(FP32))
            nc.sync.dma_start(dbg["d_dsb"], dsb[:])

    nc.compile()
    return nc


_NC_CACHE = []


def _get_program():
    if not _NC_CACHE:
        _NC_CACHE.append(_build_program())
    return _NC_CACHE[0]


def _pack_inputs(x, w_qkv, b_qkv, w_proj, b_proj):
    x2 = np.ascontiguousarray(np.asarray(x, dtype=np.float32)[0])     # [T, C]
    w_qkv = np.asarray(w_qkv, dtype=np.float32)
    b_qkv = np.asarray(b_qkv, dtype=np.float32)
    w_proj = np.asarray(w_proj, dtype=np.float32)

    xT = np.ascontiguousarray(x2.T)                                    # [C, T]
    ident = np.concatenate([np.eye(HD, dtype=np.float32)] * 2, axis=0)
    vones_np = np.ones((128, T // KT), dtype=np.float32)

    # per-query-group gather indices + transposed query slices + masks
    qidx, xqT, masks = [], [], []
    for qg in range(2):
        idx = np.concatenate(
            [np.arange(sb * QSB, (sb + 1) * QSB) for sb in SB_QG[qg]])
        qidx.append(idx)
        xqT.append(np.ascontiguousarray(x2[idx].T))                    # [C, 2048]
        mk = np.zeros((R, 128, MASK_KT * QSB), dtype=np.float32)
        for r in range(R):
            sb = SB_QG[qg][r]
            qpos = sb * QSB + np.arange(QSB)                           # [256]
            for j in range(MASK_KT):
                ktile = L_R[r] - MASK_KT + j
                kpos = ktile * KT + np.arange(KT)                      # [128]
                mk[r, :, j * QSB:(j + 1) * QSB] = np.where(
                    kpos[:, None] <= qpos[None, :], 0.0, NEG)
        masks.append(mk)

    in_maps = []
    for c in range(N_CORES):
        hg, qg = c // 2, c % 2
        heads = [HPC * hg + i for i in range(HPC)]
        qcols = np.concatenate([np.arange(h * HD, (h + 1) * HD) for h in heads])
        wq_p = np.ascontiguousarray(w_qkv[:, qcols])
        wk_p = w_qkv[:, C + qcols]
        wv_p = w_qkv[:, 2 * C + qcols]
        wkv_p = np.ascontiguousarray(np.concatenate([wk_p, wv_p], axis=1))
        bq_p = np.zeros((2, 128, 1), np.float32)
        bq_p.reshape(-1)[:HPC * HD] = b_qkv[qcols] / np.sqrt(HD)
        bkv_p = np.zeros((3, 128, 1), np.float32)
        bkv_p.reshape(-1)[:2 * HPC * HD] = np.concatenate(
            [b_qkv[C + qcols], b_qkv[2 * C + qcols]])
        wp_p = np.ascontiguousarray(
            w_proj[np.concatenate([np.arange(h * HD, (h + 1) * HD)
                                   for h in heads]), :])
        in_maps.append({
            "xT": xT, "xqT": xqT[qg], "wkv": wkv_p, "wq": wq_p, "wp": wp_p,
            "bkv": bkv_p, "bq": bq_p, "masks": masks[qg], "ident": ident,
            "vones": vones_np,
        })
    return in_maps, qidx


def kernel(x, w_qkv, b_qkv, w_proj, b_proj, _return_bass_results=False):
    nc = _get_program()
    in_maps, qidx = _pack_inputs(x, w_qkv, b_qkv, w_proj, b_proj)
    res = run_bass_kernel_spmd(nc, in_maps, core_ids=list(range(N_CORES)))
    y = np.zeros((T, C), dtype=np.float32)
    for c in range(N_CORES):
        qg = c % 2
        y[qidx[qg]] += res.results[c]["yT"].T
    y += np.asarray(b_proj, dtype=np.float32)
    out = y[None]
    if _return_bass_results:
        return out, res
    return out
